# revision 17
# baseline (speedup 1.0000x reference)
"""Round-3 Trainium2 Bass kernel for the dense transformer block.

Key structure (vs the fp32r baseline):
- bf16 matmuls everywhere (fp32 PSUM): 1 elem/cycle moving-operand streaming
  plus separate LDWEIGHTS the PE pulls ahead of in-flight matmuls.
- Host permutes the token order per core (chunk order j=0: [1,0,2,3],
  j=1: [0,1,3,2]) so every core's OWN chunks sit at permuted positions
  {1,3}: one full-sequence LN1 feeds Q, K and V; the causal-diagonal block
  of each query chunk is at a fixed key slot (qc0 -> key tiles 4..7,
  qc1 -> 12..15), masked by one shared triangular constant. Whole-block
  visibility differences between cores are {0,-1e30} bias columns folded
  into the exp activation. No fully-general mask tensors, no per-block
  DVE mask adds outside the diagonal.
- K^T, V, x, hT all stay resident in SBUF (no DRAM spill).
- Software-pipelined emission: the attention group loop emits QK one
  iteration ahead of PV, and interleaves the NEXT group's K/Q projection
  matmuls as PE filler so the tensor engine never idles while the exp
  (ACT) of the current tile is in flight -- this both hides the
  mask->exp latency and keeps the PE HAM clock-gate at 2.4 GHz.
- MLP: all of h1 = relu(fc1) first, then fc2 accumulates the full
  4096-deep contraction in PSUM (no fp32 accumulation pass in SBUF).
- Softmax denominators come from a ones-column appended to V; their
  reciprocals run on SBUF copies (reciprocal_approx_fast is wrong on HW
  for PSUM inputs) batched per pass.
"""

from contextlib import ExitStack

import numpy as np
import ml_dtypes

import concourse.bacc as bacc
import concourse.bass as bass
import concourse.tile as tile
from concourse import mybir
from concourse.bass_utils import run_bass_kernel_spmd

F32 = mybir.dt.float32
BF16 = mybir.dt.bfloat16
NPBF16 = ml_dtypes.bfloat16
P = 128
B, T, C = 4, 2048, 1024
H, D = 16, 64
DFF = 4096
TOWN = 1024
EPS = 1e-5
SCALE = D ** -0.5
NEG = -1e30

KT_C = C // P
FT_C = C // P
TT_FULL = T // P
NGROUP = H // 2
NB_OWN = TOWN // 512

Ident = mybir.ActivationFunctionType.Identity
Sqrt = mybir.ActivationFunctionType.Sqrt
Exp = mybir.ActivationFunctionType.Exp
Relu = mybir.ActivationFunctionType.Relu

# own token slices in the permuted layout
OWN = [slice(512, 1024), slice(1536, 2048)]


def _alloc(pool, n, shape, dt, prefix, **kw):
    return [
        pool.tile(list(shape), dt, tag=f"{prefix}{i}", name=f"{prefix}{i}", **kw)
        for i in range(n)
    ]


def _ln_block(nc, xs, dst, sl, g_col, b_col, eps_t, ones1, st_ps, rowp, tpool,
              bcp, prefix):
    """One 512-token LayerNorm block, feature-major. xs: 8 [P,512] bf16 APs."""
    ssum = st_ps.tile([1, 512], F32, tag="ssum", name=f"{prefix}ss")
    ssq = st_ps.tile([1, 512], F32, tag="ssq", name=f"{prefix}sq")
    for kt in range(KT_C):
        nc.tensor.matmul(ssum, ones1, xs[kt],
                         start=(kt == 0), stop=(kt == KT_C - 1))
    for kt in range(KT_C):
        sq = tpool.tile([P, 512], BF16, tag="sqt", name=f"{prefix}sqt{kt}")
        nc.vector.tensor_mul(out=sq, in0=xs[kt], in1=xs[kt])
        nc.tensor.matmul(ssq, ones1, sq,
                         start=(kt == 0), stop=(kt == KT_C - 1))
    r0 = rowp.tile([1, 512], F32, tag="r0", name=f"{prefix}mu")       # mu
    nc.scalar.mul(r0, ssum, 1.0 / C)
    r1 = rowp.tile([1, 512], F32, tag="r1", name=f"{prefix}msq")      # msq->var->rs
    nc.scalar.mul(r1, ssq, 1.0 / C)
    r2 = rowp.tile([1, 512], F32, tag="r2", name=f"{prefix}mu2")      # mu^2->std
    nc.vector.tensor_mul(out=r2, in0=r0, in1=r0)
    nc.vector.tensor_sub(out=r1, in0=r1, in1=r2)
    nc.scalar.activation(out=r2, in_=r1, func=Sqrt,
                         bias=eps_t[0:1, 0:1], scale=1.0)
    nc.vector.reciprocal_approx_fast(out=r1, in_=r2)
    mu16 = rowp.tile([1, 512], BF16, tag="mu16", name=f"{prefix}mu16")
    nc.vector.tensor_copy(out=mu16, in_=r0)
    rs16 = rowp.tile([1, 512], BF16, tag="rs16", name=f"{prefix}rs16")
    nc.vector.tensor_copy(out=rs16, in_=r1)
    mu_b = bcp.tile([P, 512], BF16, tag="mub", name=f"{prefix}mub")
    nc.gpsimd.partition_broadcast(mu_b, mu16)
    rs_b = bcp.tile([P, 512], BF16, tag="rsb", name=f"{prefix}rsb")
    nc.gpsimd.partition_broadcast(rs_b, rs16)
    for ft in range(FT_C):
        t = tpool.tile([P, 512], BF16, tag="ap", name=f"{prefix}ap{ft}")
        nc.vector.tensor_sub(out=t, in0=xs[ft], in1=mu_b)
        nc.vector.tensor_mul(out=t, in0=t, in1=rs_b)
        nc.scalar.activation(out=dst[ft][:, sl], in_=t, func=Ident,
                             bias=b_col[:, ft:ft + 1],
                             scale=g_col[:, ft:ft + 1])


def build_nc():
    nc = bacc.Bacc()
    xT_full = nc.declare_dram_parameter("xT_full", [C, T], BF16, isOutput=False)
    tri_mask = nc.declare_dram_parameter("tri_mask", [512, 512], BF16,
                                         isOutput=False)
    kbias = nc.declare_dram_parameter("kbias", [P, 8], F32, isOutput=False)
    attn_w = nc.declare_dram_parameter("attn_w", [C, 3 * C], BF16, isOutput=False)
    attn_b = nc.declare_dram_parameter("attn_b", [3 * C], F32, isOutput=False)
    proj_w = nc.declare_dram_parameter("proj_w", [C, C], BF16, isOutput=False)
    proj_b = nc.declare_dram_parameter("proj_b", [C], F32, isOutput=False)
    ln1_g = nc.declare_dram_parameter("ln1_g", [C], F32, isOutput=False)
    ln1_b = nc.declare_dram_parameter("ln1_b", [C], F32, isOutput=False)
    ln2_g = nc.declare_dram_parameter("ln2_g", [C], F32, isOutput=False)
    ln2_b = nc.declare_dram_parameter("ln2_b", [C], F32, isOutput=False)
    fc1_w = nc.declare_dram_parameter("fc1_w", [C, DFF], BF16, isOutput=False)
    fc1_b = nc.declare_dram_parameter("fc1_b", [DFF], F32, isOutput=False)
    fc2_w = nc.declare_dram_parameter("fc2_w", [DFF, C], BF16, isOutput=False)
    fc2_b = nc.declare_dram_parameter("fc2_b", [C], F32, isOutput=False)
    out = nc.declare_dram_parameter("out", [C, TOWN], F32, isOutput=True)

    with tile.TileContext(nc) as tc, ExitStack() as top:
        const = top.enter_context(tc.tile_pool(name="const", bufs=1))
        eps_t = const.tile([P, 1], F32, name="eps_t")
        nc.vector.memset(eps_t, EPS)
        ones1 = const.tile([P, 1], BF16, name="ones1")
        nc.vector.memset(ones1, 1.0)
        ones16 = const.tile([P, H], BF16, name="ones16")
        nc.vector.memset(ones16, 1.0)
        ln1g_t = const.tile([P, FT_C], F32, name="ln1g_t")
        ln1b_t = const.tile([P, FT_C], F32, name="ln1b_t")
        ln2g_t = const.tile([P, FT_C], F32, name="ln2g_t")
        ln2b_t = const.tile([P, FT_C], F32, name="ln2b_t")
        nc.sync.dma_start(out=ln1g_t, in_=ln1_g.rearrange("(f p) -> p f", p=P))
        nc.sync.dma_start(out=ln1b_t, in_=ln1_b.rearrange("(f p) -> p f", p=P))
        nc.sync.dma_start(out=ln2g_t, in_=ln2_g.rearrange("(f p) -> p f", p=P))
        nc.sync.dma_start(out=ln2b_t, in_=ln2_b.rearrange("(f p) -> p f", p=P))
        abq_t = const.tile([P, NGROUP], F32, name="abq_t")
        abk_t = const.tile([P, NGROUP], F32, name="abk_t")
        nc.sync.dma_start(out=abq_t, in_=attn_b[0:C].rearrange("(g p) -> p g", p=P))
        nc.sync.dma_start(out=abk_t,
                          in_=attn_b[C:2 * C].rearrange("(g p) -> p g", p=P))
        projb_t = const.tile([P, FT_C], F32, name="projb_t")
        nc.sync.dma_start(out=projb_t, in_=proj_b.rearrange("(f p) -> p f", p=P))
        fc2b_t = const.tile([P, FT_C], F32, name="fc2b_t")
        nc.sync.dma_start(out=fc2b_t, in_=fc2_b.rearrange("(f p) -> p f", p=P))
        fc1b_t = const.tile([P, DFF // P], F32, name="fc1b_t")
        nc.sync.dma_start(out=fc1b_t, in_=fc1_b.rearrange("(f p) -> p f", p=P))
        kb_t = const.tile([P, 8], F32, name="kb_t")
        nc.sync.dma_start(out=kb_t, in_=kbias[:, :])
        bv_bc = const.tile([P, C], F32, name="bv_bc")
        abv = attn_b[2 * C:3 * C]
        nc.sync.dma_start(
            out=bv_bc,
            in_=bass.AP(tensor=abv.tensor, offset=abv.offset,
                        ap=[[0, P]] + list(abv.ap[-1:])))

        # HAM warm-up: a burst of dummy matmuls on a memset tile flips the
        # PE clock-gate to 8/8 while the initial DMAs are still in flight.
        with ExitStack() as cw:
            warm = cw.enter_context(tc.tile_pool(name="warm", bufs=1))
            wsrc = warm.tile([P, 512], BF16, name="wsrc")
            nc.vector.memset(wsrc, 0.0)
            wps = cw.enter_context(tc.tile_pool(name="wps", bufs=1,
                                                space="PSUM"))
            wdst = wps.tile([P, 512], F32, name="wdst")
            for i in range(24):
                nc.tensor.matmul(wdst, wsrc[:, 0:P], wsrc,
                                 start=True, stop=True)

        big = top.enter_context(tc.tile_pool(name="big", bufs=1))
        qT = [big.tile([P, TOWN], BF16, tag=f"Y{i}", name=f"qT{i}")
              for i in range(NGROUP)]

        s_xt = ExitStack()      # closed after proj residual
        xtp = s_xt.enter_context(tc.tile_pool(name="xtp", bufs=1))
        # own-token x slices (residual + LN blocks 1/3), resident to phase 4
        xo = _alloc(xtp, KT_C, [P, TOWN], BF16, "xo")

        def x_pair(kt, col0):
            """[P, 2x512] strided view of xT_full: cols col0+{0,1024}."""
            base = xT_full[kt * P:(kt + 1) * P, :]
            return bass.AP(tensor=base.tensor,
                           offset=base.offset + col0,
                           ap=[list(base.ap[0]), [1024, 2], [1, 512]])

        for kt in range(KT_C):
            nc.gpsimd.dma_start(out=xo[kt].rearrange("p (b x) -> p b x", b=2),
                                in_=x_pair(kt, 512))

        s23 = ExitStack()       # hT/kT/vT: closed after attention
        hfp = s23.enter_context(tc.tile_pool(name="hfp", bufs=1))
        hT = _alloc(hfp, FT_C, [P, T], BF16, "hT")
        kvp = s23.enter_context(tc.tile_pool(name="kvp", bufs=1))
        kT = _alloc(kvp, NGROUP, [P, T], BF16, "kT")
        vT = kvp.tile([P, NGROUP, TT_FULL, 130], BF16, name="vT")

        wstream = s23.enter_context(tc.tile_pool(name="wstream", bufs=1))

        def stream_w(dram_slice, tag, name, ncols, bufs=2):
            w = wstream.tile([P, ncols], BF16, tag=tag, name=f"{name}_w",
                             bufs=bufs)
            nc.gpsimd.dma_start(out=w, in_=dram_slice)
            return w

        # ---- Phase 1+2a: LN1 (full seq) interleaved with V ----
        with ExitStack() as c1:
            st_ps = c1.enter_context(tc.tile_pool(name="st_ps", bufs=2,
                                                  space="PSUM"))
            rowp = c1.enter_context(tc.tile_pool(name="rowp", bufs=1))
            tpool = c1.enter_context(tc.tile_pool(name="tpool", bufs=2))
            lnp = c1.enter_context(tc.tile_pool(name="lnp", bufs=1))
            bcp = c1.enter_context(tc.tile_pool(name="bcp", bufs=2))
            mm_ps = c1.enter_context(
                tc.tile_pool(name="mm_ps", bufs=2, space="PSUM"))
            wvp = c1.enter_context(tc.tile_pool(name="wvp", bufs=1))
            wv_all = []
            for kt in range(KT_C):
                w = wvp.tile([P, 1024], BF16, tag=f"wv{kt}",
                             name=f"wv{kt}_w", bufs=1)
                nc.gpsimd.dma_start(
                    out=w, in_=attn_w[kt * P:(kt + 1) * P, 2 * C:2 * C + 1024])
                wv_all.append([w[:, 0:512], w[:, 512:1024]])
            def emit_v(tt):
                pss = [mm_ps.tile([P, 512], F32, tag=f"mm{nb}",
                                  name=f"vps{tt}_{nb}") for nb in range(2)]
                for kt in range(KT_C):
                    for nb in range(2):
                        nc.tensor.matmul(
                            pss[nb], hT[kt][:, tt * P:(tt + 1) * P],
                            wv_all[kt][nb], start=(kt == 0),
                            stop=(kt == KT_C - 1))
                for nb in range(2):
                    nc.vector.tensor_add(
                        out=vT[:, nb * 4:(nb + 1) * 4, tt, :].rearrange(
                            "p g (h x) -> p g h x", x=65)[:, :, :, 0:64],
                        in0=pss[nb].rearrange("p (g h d) -> p g h d",
                                              h=2, d=64),
                        in1=bv_bc[:, nb * 512:(nb + 1) * 512].rearrange(
                            "p (g h d) -> p g h d", h=2, d=64))
                nc.vector.tensor_copy(
                    out=vT[:, :, tt, 64::65].rearrange("p g h -> p g h"),
                    in_=ones16.rearrange("p (g h) -> p g h", h=2))

            # blocks 0 and 2 (non-own) preloaded as one strided DMA per kt
            x02 = _alloc(lnp, KT_C, [P, 1024], BF16, "x02")
            for kt in range(KT_C):
                nc.gpsimd.dma_start(
                    out=x02[kt].rearrange("p (b x) -> p b x", b=2),
                    in_=x_pair(kt, 0))

            def ln_inputs(nb):
                if nb == 1:
                    return [xo[kt][:, 0:512] for kt in range(KT_C)]
                if nb == 3:
                    return [xo[kt][:, 512:1024] for kt in range(KT_C)]
                half = nb // 2
                return [x02[kt][:, half * 512:(half + 1) * 512]
                        for kt in range(KT_C)]

            for nb in range(4):
                sl = slice(nb * 512, (nb + 1) * 512)
                _ln_block(nc, ln_inputs(nb), hT, sl, ln1g_t, ln1b_t, eps_t,
                          ones1, st_ps, rowp, tpool, bcp, f"lf{nb}")
                if nb >= 1:
                    for tt in range((nb - 1) * 4, nb * 4):
                        emit_v(tt)
            for tt in range(12, 16):
                emit_v(tt)

        # ---- Phase 2b/3: K/Q per group, software-pipelined into the
        #      previous group's attention ----
        attnT = [big.tile([P, TOWN], BF16, tag=f"Z{i}", name=f"attnT{i}")
                 for i in range(FT_C)]
        s3 = ExitStack()
        kq_ps = s3.enter_context(tc.tile_pool(name="kq_ps", bufs=1,
                                              space="PSUM"))
        sc_ps = s3.enter_context(tc.tile_pool(name="sc_ps", bufs=4,
                                              space="PSUM"))
        y_ps_pool = s3.enter_context(tc.tile_pool(name="y_ps", bufs=1,
                                                  space="PSUM"))
        ppool = s3.enter_context(tc.tile_pool(name="ppool", bufs=4))
        npool = s3.enter_context(tc.tile_pool(name="npool", bufs=1))
        mpool = s3.enter_context(tc.tile_pool(name="mpool", bufs=1))
        tri = _alloc(mpool, 4, [P, 512], BF16, "tri")
        for i in range(4):
            nc.gpsimd.dma_start(out=tri[i], in_=tri_mask[i * P:(i + 1) * P, :])

        def kq_gen(g):
            """Generator emitting K_g then Q_g in small steps (PE filler)."""
            wk_g = [stream_w(
                attn_w[kt * P:(kt + 1) * P, C + g * P:C + (g + 1) * P],
                f"wk{kt}", f"wk{g}_{kt}", P) for kt in range(KT_C)]
            for half in range(2):
                pss = [kq_ps.tile([P, 512], F32, tag=f"kq{nb}",
                                  name=f"kps{g}_{half}_{nb}")
                       for nb in range(2)]
                for kt in range(KT_C):
                    for nb in range(2):
                        nc.tensor.matmul(
                            pss[nb], wk_g[kt],
                            hT[kt][:, half * 1024 + nb * 512:
                                   half * 1024 + (nb + 1) * 512],
                            start=(kt == 0), stop=(kt == KT_C - 1))
                    yield
                for nb in range(2):
                    nc.vector.tensor_scalar_add(
                        out=kT[g][:, half * 1024 + nb * 512:
                                  half * 1024 + (nb + 1) * 512],
                        in0=pss[nb], scalar1=abk_t[:, g:g + 1])
                yield
            wq_g = [stream_w(
                attn_w[kt * P:(kt + 1) * P, g * P:(g + 1) * P],
                f"wq{kt}", f"wq{g}_{kt}", P) for kt in range(KT_C)]
            pss = [kq_ps.tile([P, 512], F32, tag=f"kq{nb}",
                              name=f"qps{g}_{nb}") for nb in range(2)]
            for kt in range(KT_C):
                for nb in range(NB_OWN):
                    nc.tensor.matmul(
                        pss[nb], wq_g[kt], hT[kt][:, OWN[nb]],
                        start=(kt == 0), stop=(kt == KT_C - 1))
                yield
            for nb in range(NB_OWN):
                nc.vector.tensor_scalar_add(
                    out=qT[g][:, nb * 512:(nb + 1) * 512], in0=pss[nb],
                    scalar1=abq_t[:, g:g + 1])
            yield

        def attention_group(g, filler):
            def fill(n=1):
                for _ in range(n):
                    if next(filler, None) is None:
                        break

            for qc in range(2):          # pass A: qc0 (keys 0..1023), B: qc1
                nkt = 8 if qc == 0 else 16
                y_ps = {hh: y_ps_pool.tile([65, 512], F32, tag=f"yh{hh}",
                                           name=f"y{g}_{qc}_{hh}")
                        for hh in range(2)}
                iters = [(kt, hh) for kt in range(nkt) for hh in range(2)]

                def emit_qk(kt, hh):
                    hsl = slice(64 * hh, 64 * (hh + 1))
                    sc = sc_ps.tile([P, 512], F32, tag="sc",
                                    name=f"sc{g}_{qc}_{kt}_{hh}")
                    nc.tensor.matmul(
                        sc, kT[g][hsl, kt * P:(kt + 1) * P],
                        qT[g][hsl, qc * 512:(qc + 1) * 512],
                        start=True, stop=True,
                        tile_position=(64 * hh, 0))
                    return sc

                def emit_rest(kt, hh, sc):
                    diag = range(4, 8) if qc == 0 else range(12, 16)
                    if kt in diag:
                        nc.vector.tensor_add(out=sc, in0=sc,
                                             in1=tri[kt - diag[0]])
                    pt = ppool.tile([P, 512], BF16, tag="pt",
                                    name=f"p{g}_{qc}_{kt}_{hh}")
                    nc.scalar.activation(
                        out=pt, in_=sc, func=Exp, scale=SCALE,
                        bias=kb_t[:, qc * 4 + kt // 4:qc * 4 + kt // 4 + 1])
                    nc.tensor.matmul(
                        y_ps[hh], vT[:, g, kt, 65 * hh:65 * (hh + 1)], pt,
                        start=(kt == 0), stop=(kt == nkt - 1))

                LOOK = 3
                scq = [emit_qk(*iters[i]) for i in range(LOOK)]
                for i, it in enumerate(iters):
                    if i + LOOK < len(iters):
                        scq.append(emit_qk(*iters[i + LOOK]))
                    if i % 2 == 0:
                        fill(1)
                    emit_rest(*it, scq.pop(0))

                # normalize this pass: denominators -> SBUF, recip, scale
                for hh in range(2):
                    dn = npool.tile([1, 512], F32, tag=f"dn{hh}",
                                    name=f"dn{g}_{qc}_{hh}")
                    nc.vector.tensor_copy(out=dn, in_=y_ps[hh][64:65, :])
                    rc = npool.tile([1, 512], F32, tag=f"rc{hh}",
                                    name=f"rc{g}_{qc}_{hh}")
                    nc.vector.reciprocal_approx_fast(out=rc, in_=dn)
                    r16 = npool.tile([1, 512], BF16, tag=f"r16{hh}",
                                     name=f"r16{g}_{qc}_{hh}")
                    nc.vector.tensor_copy(out=r16, in_=rc)
                    rb = npool.tile([64, 512], BF16, tag=f"rb{hh}",
                                    name=f"rb{g}_{qc}_{hh}")
                    nc.gpsimd.partition_broadcast(rb, r16)
                    nc.vector.tensor_mul(
                        out=attnT[g][64 * hh:64 * (hh + 1),
                                     qc * 512:(qc + 1) * 512],
                        in0=y_ps[hh][0:64, :], in1=rb)
                fill(1)

        fillers = [kq_gen(g) for g in range(NGROUP)]
        # K_0/Q_0 up front
        for _ in fillers[0]:
            pass
        for g in range(NGROUP):
            filler = fillers[g + 1] if g + 1 < NGROUP else iter(())
            attention_group(g, filler)
            for _ in filler:        # drain leftovers
                pass
        s3.close()
        s23.close()

        # ---- Phase 4: proj + residual + LN2 ----
        x2T = [big.tile([P, TOWN], BF16, tag=f"Y{i}", name=f"x2T{i}")
               for i in range(FT_C)]
        h2T = [big.tile([P, TOWN], BF16, tag=f"Z{i}", name=f"h2T{i}")
               for i in range(FT_C)]
        with ExitStack() as c4:
            w4 = c4.enter_context(tc.tile_pool(name="w4", bufs=1))
            pw = _alloc(w4, KT_C, [P, C], BF16, "pw")
            for kt in range(KT_C):
                nc.gpsimd.dma_start(out=pw[kt],
                                    in_=proj_w[kt * P:(kt + 1) * P, :])
            xop = c4.enter_context(tc.tile_pool(name="xop", bufs=3))
            mm_ps4 = c4.enter_context(
                tc.tile_pool(name="mm_ps4", bufs=2, space="PSUM"))

            for ft in range(FT_C):
                pss = [mm_ps4.tile([P, 512], F32, tag=f"mm{nb}",
                                   name=f"prj{ft}_{nb}")
                       for nb in range(NB_OWN)]
                for kt in range(KT_C):
                    for nb in range(NB_OWN):
                        nc.tensor.matmul(
                            pss[nb], pw[kt][:, ft * P:(ft + 1) * P],
                            attnT[kt][:, nb * 512:(nb + 1) * 512],
                            start=(kt == 0), stop=(kt == KT_C - 1))
                for nb in range(NB_OWN):
                    sl = slice(nb * 512, (nb + 1) * 512)
                    t = xop.tile([P, 512], F32, tag="t4", name=f"t4{ft}_{nb}")
                    nc.vector.tensor_scalar_add(out=t, in0=pss[nb],
                                                scalar1=projb_t[:, ft:ft + 1])
                    nc.vector.tensor_add(out=x2T[ft][:, sl], in0=t,
                                         in1=xo[ft][:, sl])

            st4 = c4.enter_context(tc.tile_pool(name="st4", bufs=2,
                                                space="PSUM"))
            rowp4 = c4.enter_context(tc.tile_pool(name="rowp4", bufs=1))
            tpool4 = c4.enter_context(tc.tile_pool(name="tpool4", bufs=2))
            bcp4 = c4.enter_context(tc.tile_pool(name="bcp4", bufs=2))
            for nb in range(NB_OWN):
                sl = slice(nb * 512, (nb + 1) * 512)
                _ln_block(nc, [x2T[kt][:, sl] for kt in range(KT_C)], h2T,
                          sl, ln2g_t, ln2b_t, eps_t, ones1, st4, rowp4,
                          tpool4, bcp4, f"l2{nb}")
        s_xt.close()

        # ---- Phase 5: MLP in two d_ff halves (SBUF-pressure): per half,
        #      h1 = relu(fc1) for 2048 dff rows, then fc2 accumulated
        #      16-deep in PSUM; halves combined in an fp32 SBUF accum ----
        NKT_H = DFF // P // 2   # 16 dff tiles per half
        with ExitStack() as c5:
            h1_pool = c5.enter_context(tc.tile_pool(name="h1_pool", bufs=1))
            oaccp = c5.enter_context(tc.tile_pool(name="oaccp", bufs=1))
            oacc = _alloc(oaccp, FT_C, [P, TOWN], F32, "oacc")
            w52 = c5.enter_context(tc.tile_pool(name="w52", bufs=1))
            w51 = c5.enter_context(tc.tile_pool(name="w51", bufs=1))
            mm_ps5 = c5.enter_context(
                tc.tile_pool(name="mm_ps5", bufs=2, space="PSUM"))
            mm_ps6 = c5.enter_context(
                tc.tile_pool(name="mm_ps6", bufs=2, space="PSUM"))
            opool = c5.enter_context(tc.tile_pool(name="opool", bufs=2))
            for dh in range(2):
                h1 = _alloc(h1_pool, NKT_H, [P, TOWN], BF16, "h1")
                w2 = [w52.tile([P, C], BF16, tag=f"w2_{i}", name=f"w2{dh}_{i}",
                               bufs=1)
                      for i in range(NKT_H)]
                for i in range(NKT_H):
                    d_ = dh * NKT_H + i
                    nc.gpsimd.dma_start(
                        out=w2[i], in_=fc2_w[d_ * P:(d_ + 1) * P, :])
                for dc in range(4):
                    w1c = [w51.tile([P, 512], BF16, tag=f"w1c{i}",
                                    name=f"w1c{dh}_{dc}_{i}", bufs=2)
                           for i in range(KT_C)]
                    for kt in range(KT_C):
                        nc.gpsimd.dma_start(
                            out=w1c[kt],
                            in_=fc1_w[kt * P:(kt + 1) * P,
                                      (dh * 4 + dc) * 512:
                                      (dh * 4 + dc + 1) * 512])
                    for m8 in range(4):
                        pss = [mm_ps5.tile([P, 512], F32, tag=f"m5{nb}",
                                           name=f"f1{dh}_{dc}_{m8}_{nb}")
                               for nb in range(NB_OWN)]
                        for kt in range(KT_C):
                            for nb in range(NB_OWN):
                                nc.tensor.matmul(
                                    pss[nb], w1c[kt][:, m8 * P:(m8 + 1) * P],
                                    h2T[kt][:, nb * 512:(nb + 1) * 512],
                                    start=(kt == 0), stop=(kt == KT_C - 1))
                        d_ = dh * NKT_H + dc * 4 + m8
                        for nb in range(NB_OWN):
                            nc.scalar.activation(
                                out=h1[dc * 4 + m8][:, nb * 512:(nb + 1) * 512],
                                in_=pss[nb], func=Relu,
                                bias=fc1b_t[:, d_:d_ + 1], scale=1.0)
                for ft in range(FT_C):
                    pss = [mm_ps6.tile([P, 512], F32, tag=f"m6{nb}",
                                       name=f"f2{dh}_{ft}_{nb}")
                           for nb in range(NB_OWN)]
                    for kt in range(NKT_H):
                        for nb in range(NB_OWN):
                            nc.tensor.matmul(
                                pss[nb], w2[kt][:, ft * P:(ft + 1) * P],
                                h1[kt][:, nb * 512:(nb + 1) * 512],
                                start=(kt == 0), stop=(kt == NKT_H - 1))
                    if dh == 0:
                        for nb in range(NB_OWN):
                            sl = slice(nb * 512, (nb + 1) * 512)
                            nc.vector.tensor_copy(out=oacc[ft][:, sl],
                                                  in_=pss[nb])
                    else:
                        o = opool.tile([P, TOWN], F32, tag="o", name=f"o{ft}")
                        for nb in range(NB_OWN):
                            sl = slice(nb * 512, (nb + 1) * 512)
                            nc.vector.tensor_add(out=o[:, sl],
                                                 in0=pss[nb],
                                                 in1=oacc[ft][:, sl])
                            nc.vector.tensor_scalar_add(
                                out=o[:, sl], in0=o[:, sl],
                                scalar1=fc2b_t[:, ft:ft + 1])
                            nc.vector.tensor_add(out=o[:, sl], in0=o[:, sl],
                                                 in1=x2T[ft][:, sl])
                        nc.sync.dma_start(out=out[ft * P:(ft + 1) * P, :],
                                          in_=o)

    nc.compile()
    return nc


_NC_CACHE = None


def _get_nc():
    global _NC_CACHE
    if _NC_CACHE is None:
        _NC_CACHE = build_nc()
    return _NC_CACHE


# permuted chunk order per core flavor j (position -> source chunk)
_PERM = {0: [1, 0, 2, 3], 1: [0, 1, 3, 2]}
_OWN_POS = (1, 3)


def _make_tri():
    # [512 keys, 512 q] lower-triangular (key visible iff k <= q), packed the
    # same way as the score tiles: row-block i holds key tiles (2i, 2i+1).
    k = np.arange(512, dtype=np.int64)[:, None]
    q = np.arange(512, dtype=np.int64)[None, :]
    m = np.where(k <= q, 0.0, NEG).astype(np.float32)   # [512k, 512q]
    return np.ascontiguousarray(m, dtype=NPBF16)


def _make_kbias(j):
    kb = np.zeros((P, 8), np.float32)
    if j == 0:
        kb[:, 0] = NEG          # qc0, key slot 0 (= chunk 1) invisible
    else:
        kb[:, 6] = NEG          # qc1, key slot 2 (= chunk 3) invisible
    return kb


def _run(inputs, trace=False):
    nc = _get_nc()
    xs = {k: np.ascontiguousarray(np.asarray(v), dtype=np.float32)
          for k, v in inputs.items()}
    x = xs["x"]
    b16 = lambda a: np.ascontiguousarray(a, dtype=NPBF16)
    attn_w16 = b16(xs["attn_w"])
    proj_w16 = b16(xs["proj_w"])
    fc1_w16 = b16(xs["fc1_w"])
    fc2_w16 = b16(xs["fc2_w"])
    tri = _make_tri()
    kbs = {j: _make_kbias(j) for j in range(2)}
    in_maps = []
    for c in range(8):
        b, j = divmod(c, 2)
        perm = _PERM[j]
        xT = x[b].T
        xT_perm = b16(np.concatenate([xT[:, p * 512:(p + 1) * 512]
                                      for p in perm], axis=1))
        in_maps.append({
            "xT_full": xT_perm,
            "tri_mask": tri,
            "kbias": kbs[j],
            "attn_w": attn_w16, "attn_b": xs["attn_b"],
            "proj_w": proj_w16, "proj_b": xs["proj_b"],
            "ln1_g": xs["ln1_g"], "ln1_b": xs["ln1_b"],
            "ln2_g": xs["ln2_g"], "ln2_b": xs["ln2_b"],
            "fc1_w": fc1_w16, "fc1_b": xs["fc1_b"],
            "fc2_w": fc2_w16, "fc2_b": xs["fc2_b"],
        })
    res = run_bass_kernel_spmd(nc, in_maps, list(range(8)), trace=trace)
    full = np.empty((B, T, C), dtype=np.float32)
    for c in range(8):
        b, j = divmod(c, 2)
        perm = _PERM[j]
        cl, ch = perm[_OWN_POS[0]], perm[_OWN_POS[1]]
        o = res.results[c]["out"]            # [C, TOWN] feature-major
        full[b, cl * 512:(cl + 1) * 512] = o[:, 0:512].T
        full[b, ch * 512:(ch + 1) * 512] = o[:, 512:1024].T
    return full, res.exec_time_ns


def kernel(**inputs):
    out, _ = _run(inputs, trace=False)
    return out


# revision 20
# speedup vs baseline: 1.0718x; 1.0718x over previous
"""Round-3 Trainium2 Bass kernel for the dense transformer block.

Key structure (vs the fp32r baseline):
- bf16 matmuls everywhere (fp32 PSUM): 1 elem/cycle moving-operand streaming
  plus separate LDWEIGHTS the PE pulls ahead of in-flight matmuls.
- Host permutes the token order per core (chunk order j=0: [1,0,2,3],
  j=1: [0,1,3,2]) so every core's OWN chunks sit at permuted positions
  {1,3}: one full-sequence LN1 feeds Q, K and V; the causal-diagonal block
  of each query chunk is at a fixed key slot (qc0 -> key tiles 4..7,
  qc1 -> 12..15), masked by one shared triangular constant. Whole-block
  visibility differences between cores are {0,-1e30} bias columns folded
  into the exp activation. No fully-general mask tensors, no per-block
  DVE mask adds outside the diagonal.
- K^T, V, x, hT all stay resident in SBUF (no DRAM spill).
- Software-pipelined emission: the attention group loop emits QK one
  iteration ahead of PV, and interleaves the NEXT group's K/Q projection
  matmuls as PE filler so the tensor engine never idles while the exp
  (ACT) of the current tile is in flight -- this both hides the
  mask->exp latency and keeps the PE HAM clock-gate at 2.4 GHz.
- MLP: all of h1 = relu(fc1) first, then fc2 accumulates the full
  4096-deep contraction in PSUM (no fp32 accumulation pass in SBUF).
- Softmax denominators come from a ones-column appended to V; their
  reciprocals run on SBUF copies (reciprocal_approx_fast is wrong on HW
  for PSUM inputs) batched per pass.
"""

from contextlib import ExitStack

import numpy as np
import ml_dtypes

import concourse.bacc as bacc
import concourse.bass as bass
import concourse.tile as tile
from concourse import mybir
from concourse.bass_utils import run_bass_kernel_spmd

F32 = mybir.dt.float32
BF16 = mybir.dt.bfloat16
NPBF16 = ml_dtypes.bfloat16
P = 128
B, T, C = 4, 2048, 1024
H, D = 16, 64
DFF = 4096
TOWN = 1024
EPS = 1e-5
SCALE = D ** -0.5
NEG = -1e30

KT_C = C // P
FT_C = C // P
TT_FULL = T // P
NGROUP = H // 2
NB_OWN = TOWN // 512

Ident = mybir.ActivationFunctionType.Identity
Sqrt = mybir.ActivationFunctionType.Sqrt
Exp = mybir.ActivationFunctionType.Exp
Relu = mybir.ActivationFunctionType.Relu

# own token slices in the permuted layout
OWN = [slice(512, 1024), slice(1536, 2048)]


def _alloc(pool, n, shape, dt, prefix, **kw):
    return [
        pool.tile(list(shape), dt, tag=f"{prefix}{i}", name=f"{prefix}{i}", **kw)
        for i in range(n)
    ]


def _ln_block(nc, xs, dst, sl, g_col, b_col, eps_t, ones1, st_ps, rowp, tpool,
              bcp, prefix):
    """One 512-token LayerNorm block, feature-major. xs: 8 [P,512] bf16 APs."""
    ssum = st_ps.tile([1, 512], F32, tag="ssum", name=f"{prefix}ss")
    ssq = st_ps.tile([1, 512], F32, tag="ssq", name=f"{prefix}sq")
    for kt in range(KT_C):
        nc.tensor.matmul(ssum, ones1, xs[kt],
                         start=(kt == 0), stop=(kt == KT_C - 1))
    for kt in range(KT_C):
        sq = tpool.tile([P, 512], BF16, tag="sqt", name=f"{prefix}sqt{kt}")
        nc.vector.tensor_mul(out=sq, in0=xs[kt], in1=xs[kt])
        nc.tensor.matmul(ssq, ones1, sq,
                         start=(kt == 0), stop=(kt == KT_C - 1))
    r0 = rowp.tile([1, 512], F32, tag="r0", name=f"{prefix}mu")       # mu
    nc.scalar.mul(r0, ssum, 1.0 / C)
    r1 = rowp.tile([1, 512], F32, tag="r1", name=f"{prefix}msq")      # msq->var->rs
    nc.scalar.mul(r1, ssq, 1.0 / C)
    r2 = rowp.tile([1, 512], F32, tag="r2", name=f"{prefix}mu2")      # mu^2->std
    nc.vector.tensor_mul(out=r2, in0=r0, in1=r0)
    nc.vector.tensor_sub(out=r1, in0=r1, in1=r2)
    nc.scalar.activation(out=r2, in_=r1, func=Sqrt,
                         bias=eps_t[0:1, 0:1], scale=1.0)
    nc.vector.reciprocal_approx_fast(out=r1, in_=r2)
    mu16 = rowp.tile([1, 512], BF16, tag="mu16", name=f"{prefix}mu16")
    nc.vector.tensor_copy(out=mu16, in_=r0)
    rs16 = rowp.tile([1, 512], BF16, tag="rs16", name=f"{prefix}rs16")
    nc.vector.tensor_copy(out=rs16, in_=r1)
    mu_b = bcp.tile([P, 512], BF16, tag="mub", name=f"{prefix}mub")
    nc.gpsimd.partition_broadcast(mu_b, mu16)
    rs_b = bcp.tile([P, 512], BF16, tag="rsb", name=f"{prefix}rsb")
    nc.gpsimd.partition_broadcast(rs_b, rs16)
    for ft in range(FT_C):
        t = tpool.tile([P, 512], BF16, tag="ap", name=f"{prefix}ap{ft}")
        nc.vector.tensor_sub(out=t, in0=xs[ft], in1=mu_b)
        nc.vector.tensor_mul(out=t, in0=t, in1=rs_b)
        nc.scalar.activation(out=dst[ft][:, sl], in_=t, func=Ident,
                             bias=b_col[:, ft:ft + 1],
                             scale=g_col[:, ft:ft + 1])


def build_nc():
    nc = bacc.Bacc()
    xT_full = nc.declare_dram_parameter("xT_full", [C, T], BF16, isOutput=False)
    tri_mask = nc.declare_dram_parameter("tri_mask", [512, 512], BF16,
                                         isOutput=False)
    kbias = nc.declare_dram_parameter("kbias", [P, 8], F32, isOutput=False)
    attn_w = nc.declare_dram_parameter("attn_w", [C, 3 * C], BF16, isOutput=False)
    attn_b = nc.declare_dram_parameter("attn_b", [3 * C], F32, isOutput=False)
    proj_w = nc.declare_dram_parameter("proj_w", [C, C], BF16, isOutput=False)
    proj_b = nc.declare_dram_parameter("proj_b", [C], F32, isOutput=False)
    ln1_g = nc.declare_dram_parameter("ln1_g", [C], F32, isOutput=False)
    ln1_b = nc.declare_dram_parameter("ln1_b", [C], F32, isOutput=False)
    ln2_g = nc.declare_dram_parameter("ln2_g", [C], F32, isOutput=False)
    ln2_b = nc.declare_dram_parameter("ln2_b", [C], F32, isOutput=False)
    fc1_w = nc.declare_dram_parameter("fc1_w", [C, DFF], BF16, isOutput=False)
    fc1_b = nc.declare_dram_parameter("fc1_b", [DFF], F32, isOutput=False)
    fc2_w = nc.declare_dram_parameter("fc2_w", [DFF, C], BF16, isOutput=False)
    fc2_b = nc.declare_dram_parameter("fc2_b", [C], F32, isOutput=False)
    out = nc.declare_dram_parameter("out", [C, TOWN], F32, isOutput=True)

    with tile.TileContext(nc) as tc, ExitStack() as top:
        const = top.enter_context(tc.tile_pool(name="const", bufs=1))
        eps_t = const.tile([P, 1], F32, name="eps_t")
        nc.vector.memset(eps_t, EPS)
        ones1 = const.tile([P, 1], BF16, name="ones1")
        nc.vector.memset(ones1, 1.0)
        ones16 = const.tile([P, H], BF16, name="ones16")
        nc.vector.memset(ones16, 1.0)
        ln1g_t = const.tile([P, FT_C], F32, name="ln1g_t")
        ln1b_t = const.tile([P, FT_C], F32, name="ln1b_t")
        ln2g_t = const.tile([P, FT_C], F32, name="ln2g_t")
        ln2b_t = const.tile([P, FT_C], F32, name="ln2b_t")
        nc.sync.dma_start(out=ln1g_t, in_=ln1_g.rearrange("(f p) -> p f", p=P))
        nc.sync.dma_start(out=ln1b_t, in_=ln1_b.rearrange("(f p) -> p f", p=P))
        nc.sync.dma_start(out=ln2g_t, in_=ln2_g.rearrange("(f p) -> p f", p=P))
        nc.sync.dma_start(out=ln2b_t, in_=ln2_b.rearrange("(f p) -> p f", p=P))
        abq_t = const.tile([P, NGROUP], F32, name="abq_t")
        abk_t = const.tile([P, NGROUP], F32, name="abk_t")
        nc.sync.dma_start(out=abq_t, in_=attn_b[0:C].rearrange("(g p) -> p g", p=P))
        nc.sync.dma_start(out=abk_t,
                          in_=attn_b[C:2 * C].rearrange("(g p) -> p g", p=P))
        projb_t = const.tile([P, FT_C], F32, name="projb_t")
        nc.sync.dma_start(out=projb_t, in_=proj_b.rearrange("(f p) -> p f", p=P))
        fc2b_t = const.tile([P, FT_C], F32, name="fc2b_t")
        nc.sync.dma_start(out=fc2b_t, in_=fc2_b.rearrange("(f p) -> p f", p=P))
        fc1b_t = const.tile([P, DFF // P], F32, name="fc1b_t")
        nc.sync.dma_start(out=fc1b_t, in_=fc1_b.rearrange("(f p) -> p f", p=P))
        kb_t = const.tile([P, 8], F32, name="kb_t")
        nc.sync.dma_start(out=kb_t, in_=kbias[:, :])
        bv_bc = const.tile([P, C], F32, name="bv_bc")
        abv = attn_b[2 * C:3 * C]
        nc.sync.dma_start(
            out=bv_bc,
            in_=bass.AP(tensor=abv.tensor, offset=abv.offset,
                        ap=[[0, P]] + list(abv.ap[-1:])))

        # HAM warm-up: a burst of dummy matmuls on a memset tile flips the
        # PE clock-gate to 8/8 while the initial DMAs are still in flight.
        with ExitStack() as cw:
            warm = cw.enter_context(tc.tile_pool(name="warm", bufs=1))
            wsrc = warm.tile([P, 512], BF16, name="wsrc")
            nc.vector.memset(wsrc, 0.0)
            wps = cw.enter_context(tc.tile_pool(name="wps", bufs=1,
                                                space="PSUM"))
            wdst = wps.tile([P, 512], F32, name="wdst")
            for i in range(24):
                nc.tensor.matmul(wdst, wsrc[:, 0:P], wsrc,
                                 start=True, stop=True)

        big = top.enter_context(tc.tile_pool(name="big", bufs=1))
        qT = [big.tile([P, TOWN], BF16, tag=f"Y{i}", name=f"qT{i}")
              for i in range(NGROUP)]

        s_xt = ExitStack()      # closed after proj residual
        xtp = s_xt.enter_context(tc.tile_pool(name="xtp", bufs=1))
        # own-token x slices (residual + LN blocks 1/3), resident to phase 4
        xo = _alloc(xtp, KT_C, [P, TOWN], BF16, "xo")

        def x_pair(kt, col0):
            """[P, 2x512] strided view of xT_full: cols col0+{0,1024}."""
            base = xT_full[kt * P:(kt + 1) * P, :]
            return bass.AP(tensor=base.tensor,
                           offset=base.offset + col0,
                           ap=[list(base.ap[0]), [1024, 2], [1, 512]])

        for kt in range(KT_C):
            nc.gpsimd.dma_start(out=xo[kt].rearrange("p (b x) -> p b x", b=2),
                                in_=x_pair(kt, 512))

        s23 = ExitStack()       # hT/kT/vT: closed after attention
        hfp = s23.enter_context(tc.tile_pool(name="hfp", bufs=1))
        hT = _alloc(hfp, FT_C, [P, T], BF16, "hT")
        kvp = s23.enter_context(tc.tile_pool(name="kvp", bufs=1))
        kT = _alloc(kvp, NGROUP, [P, T], BF16, "kT")
        vT = kvp.tile([P, NGROUP, TT_FULL, 130], BF16, name="vT")

        wstream = s23.enter_context(tc.tile_pool(name="wstream", bufs=1))

        def stream_w(dram_slice, tag, name, ncols, bufs=2):
            w = wstream.tile([P, ncols], BF16, tag=tag, name=f"{name}_w",
                             bufs=bufs)
            nc.gpsimd.dma_start(out=w, in_=dram_slice)
            return w

        wvp = s23.enter_context(tc.tile_pool(name="wvp", bufs=1))
        wv_all = []
        for kt in range(KT_C):
            w = wvp.tile([P, 1024], BF16, tag=f"wv{kt}",
                         name=f"wv{kt}_w", bufs=1)
            nc.gpsimd.dma_start(
                out=w, in_=attn_w[kt * P:(kt + 1) * P, 2 * C:2 * C + 1024])
            wv_all.append([w[:, 0:512], w[:, 512:1024]])

        # ---- Phase 1+2a: LN1 (full seq) interleaved with V ----
        with ExitStack() as c1:
            st_ps = c1.enter_context(tc.tile_pool(name="st_ps", bufs=2,
                                                  space="PSUM"))
            rowp = c1.enter_context(tc.tile_pool(name="rowp", bufs=1))
            tpool = c1.enter_context(tc.tile_pool(name="tpool", bufs=2))
            lnp = c1.enter_context(tc.tile_pool(name="lnp", bufs=1))
            bcp = c1.enter_context(tc.tile_pool(name="bcp", bufs=2))
            mm_ps = c1.enter_context(
                tc.tile_pool(name="mm_ps", bufs=2, space="PSUM"))
            def emit_v(tt):
                pss = mm_ps.tile([P, 512], F32, tag="mm0",
                                 name=f"vps{tt}_0")
                for kt in range(KT_C):
                    nc.tensor.matmul(
                        pss, hT[kt][:, tt * P:(tt + 1) * P],
                        wv_all[kt][0], start=(kt == 0),
                        stop=(kt == KT_C - 1))
                nc.vector.tensor_add(
                    out=vT[:, 0:4, tt, :].rearrange(
                        "p g (h x) -> p g h x", x=65)[:, :, :, 0:64],
                    in0=pss.rearrange("p (g h d) -> p g h d", h=2, d=64),
                    in1=bv_bc[:, 0:512].rearrange(
                        "p (g h d) -> p g h d", h=2, d=64))
                nc.vector.tensor_copy(
                    out=vT[:, :, tt, 64::65].rearrange("p g h -> p g h"),
                    in_=ones16.rearrange("p (g h) -> p g h", h=2))

            # blocks 0 and 2 (non-own) preloaded as one strided DMA per kt
            x02 = _alloc(lnp, KT_C, [P, 1024], BF16, "x02")
            for kt in range(KT_C):
                nc.gpsimd.dma_start(
                    out=x02[kt].rearrange("p (b x) -> p b x", b=2),
                    in_=x_pair(kt, 0))

            def ln_inputs(nb):
                if nb == 1:
                    return [xo[kt][:, 0:512] for kt in range(KT_C)]
                if nb == 3:
                    return [xo[kt][:, 512:1024] for kt in range(KT_C)]
                half = nb // 2
                return [x02[kt][:, half * 512:(half + 1) * 512]
                        for kt in range(KT_C)]

            for nb in range(4):
                sl = slice(nb * 512, (nb + 1) * 512)
                _ln_block(nc, ln_inputs(nb), hT, sl, ln1g_t, ln1b_t, eps_t,
                          ones1, st_ps, rowp, tpool, bcp, f"lf{nb}")
                if nb >= 1:
                    for tt in range((nb - 1) * 4, nb * 4):
                        emit_v(tt)
            for tt in range(12, 16):
                emit_v(tt)

        # ---- Phase 2b/3: K/Q per group, software-pipelined into the
        #      previous group's attention ----
        attnT = [big.tile([P, TOWN], BF16, tag=f"Z{i}", name=f"attnT{i}")
                 for i in range(FT_C)]
        s3 = ExitStack()
        kq_ps = s3.enter_context(tc.tile_pool(name="kq_ps", bufs=1,
                                              space="PSUM"))
        sc_ps = s3.enter_context(tc.tile_pool(name="sc_ps", bufs=2,
                                              space="PSUM"))
        y_ps_pool = s3.enter_context(tc.tile_pool(name="y_ps", bufs=1,
                                                  space="PSUM"))
        ppool = s3.enter_context(tc.tile_pool(name="ppool", bufs=4))
        npool = s3.enter_context(tc.tile_pool(name="npool", bufs=1))
        mpool = s3.enter_context(tc.tile_pool(name="mpool", bufs=1))
        tri = _alloc(mpool, 4, [P, 512], BF16, "tri")
        for i in range(4):
            nc.gpsimd.dma_start(out=tri[i], in_=tri_mask[i * P:(i + 1) * P, :])

        def kq_gen(g):
            """Generator emitting K_g then Q_g in small steps (PE filler)."""
            wk_g = [stream_w(
                attn_w[kt * P:(kt + 1) * P, C + g * P:C + (g + 1) * P],
                f"wk{kt}", f"wk{g}_{kt}", P) for kt in range(KT_C)]
            for half in range(2):
                pss = [kq_ps.tile([P, 512], F32, tag=f"kq{nb}",
                                  name=f"kps{g}_{half}_{nb}")
                       for nb in range(2)]
                for kt in range(KT_C):
                    for nb in range(2):
                        nc.tensor.matmul(
                            pss[nb], wk_g[kt],
                            hT[kt][:, half * 1024 + nb * 512:
                                   half * 1024 + (nb + 1) * 512],
                            start=(kt == 0), stop=(kt == KT_C - 1))
                    yield
                for nb in range(2):
                    nc.vector.tensor_scalar_add(
                        out=kT[g][:, half * 1024 + nb * 512:
                                  half * 1024 + (nb + 1) * 512],
                        in0=pss[nb], scalar1=abk_t[:, g:g + 1])
                yield
            wq_g = [stream_w(
                attn_w[kt * P:(kt + 1) * P, g * P:(g + 1) * P],
                f"wq{kt}", f"wq{g}_{kt}", P) for kt in range(KT_C)]
            pss = [kq_ps.tile([P, 512], F32, tag=f"kq{nb}",
                              name=f"qps{g}_{nb}") for nb in range(2)]
            for kt in range(KT_C):
                for nb in range(NB_OWN):
                    nc.tensor.matmul(
                        pss[nb], wq_g[kt], hT[kt][:, OWN[nb]],
                        start=(kt == 0), stop=(kt == KT_C - 1))
                yield
            for nb in range(NB_OWN):
                nc.vector.tensor_scalar_add(
                    out=qT[g][:, nb * 512:(nb + 1) * 512], in0=pss[nb],
                    scalar1=abq_t[:, g:g + 1])
            yield

        def v1_gen(tts):
            """V for head groups 4..7 (nb=1), a few token tiles per call."""
            for tt in tts:
                pss = kq_ps.tile([P, 512], F32, tag="kq0", name=f"vps{tt}_1")
                for kt in range(KT_C):
                    nc.tensor.matmul(
                        pss, hT[kt][:, tt * P:(tt + 1) * P],
                        wv_all[kt][1], start=(kt == 0),
                        stop=(kt == KT_C - 1))
                    if kt % 4 == 3:
                        yield
                nc.vector.tensor_add(
                    out=vT[:, 4:8, tt, :].rearrange(
                        "p g (h x) -> p g h x", x=65)[:, :, :, 0:64],
                    in0=pss.rearrange("p (g h d) -> p g h d", h=2, d=64),
                    in1=bv_bc[:, 512:1024].rearrange(
                        "p (g h d) -> p g h d", h=2, d=64))
                yield

        def attention_group(g, filler):
            def fill(n=1):
                for _ in range(n):
                    if next(filler, None) is None:
                        break

            for qc in range(2):          # pass A: qc0 (k2<4), pass B: qc1
                nk2 = 4 if qc == 0 else 8
                y_ps = {hh: y_ps_pool.tile([65, 512], F32, tag=f"yh{hh}",
                                           name=f"y{g}_{qc}_{hh}")
                        for hh in range(2)}
                iters = [(k2, hh) for k2 in range(nk2) for hh in range(2)]

                def emit_qk(k2, hh):
                    hsl = slice(64 * hh, 64 * (hh + 1))
                    sc = sc_ps.tile([P, 1024], F32, tag="sc",
                                    name=f"sc{g}_{qc}_{k2}_{hh}")
                    for j in range(2):
                        kt = 2 * k2 + j
                        nc.tensor.matmul(
                            sc[:, j * 512:(j + 1) * 512],
                            kT[g][hsl, kt * P:(kt + 1) * P],
                            qT[g][hsl, qc * 512:(qc + 1) * 512],
                            start=True, stop=True,
                            tile_position=(64 * hh, 0))
                    return sc

                def emit_rest(k2, hh, sc):
                    diag = (2, 3) if qc == 0 else (6, 7)
                    if k2 in diag:
                        for j in range(2):
                            nc.vector.tensor_add(
                                out=sc[:, j * 512:(j + 1) * 512],
                                in0=sc[:, j * 512:(j + 1) * 512],
                                in1=tri[2 * (k2 - diag[0]) + j])
                    pt = ppool.tile([P, 1024], BF16, tag="pt",
                                    name=f"p{g}_{qc}_{k2}_{hh}")
                    nc.scalar.activation(
                        out=pt, in_=sc, func=Exp, scale=SCALE,
                        bias=kb_t[:, qc * 4 + k2 // 2:qc * 4 + k2 // 2 + 1])
                    for j in range(2):
                        kt = 2 * k2 + j
                        nc.tensor.matmul(
                            y_ps[hh], vT[:, g, kt, 65 * hh:65 * (hh + 1)],
                            pt[:, j * 512:(j + 1) * 512],
                            start=(kt == 0), stop=(kt == 2 * nk2 - 1))

                sc_prev = emit_qk(*iters[0])
                for i, it in enumerate(iters):
                    if i + 1 < len(iters):
                        sc_next = emit_qk(*iters[i + 1])
                    fill(2)
                    emit_rest(*it, sc_prev)
                    if i + 1 < len(iters):
                        sc_prev = sc_next

                # normalize this pass: denominators -> SBUF, recip, scale
                for hh in range(2):
                    dn = npool.tile([1, 512], F32, tag=f"dn{hh}",
                                    name=f"dn{g}_{qc}_{hh}")
                    nc.vector.tensor_copy(out=dn, in_=y_ps[hh][64:65, :])
                    rc = npool.tile([1, 512], F32, tag=f"rc{hh}",
                                    name=f"rc{g}_{qc}_{hh}")
                    nc.vector.reciprocal_approx_fast(out=rc, in_=dn)
                    r16 = npool.tile([1, 512], BF16, tag=f"r16{hh}",
                                     name=f"r16{g}_{qc}_{hh}")
                    nc.vector.tensor_copy(out=r16, in_=rc)
                    rb = npool.tile([64, 512], BF16, tag=f"rb{hh}",
                                    name=f"rb{g}_{qc}_{hh}")
                    nc.gpsimd.partition_broadcast(rb, r16)
                    nc.vector.tensor_mul(
                        out=attnT[g][64 * hh:64 * (hh + 1),
                                     qc * 512:(qc + 1) * 512],
                        in0=y_ps[hh][0:64, :], in1=rb)
                fill(1)

        from itertools import chain
        # K_0/Q_0 up front
        for _ in kq_gen(0):
            pass
        for g in range(NGROUP):
            parts = []
            if g + 1 < NGROUP:
                parts.append(kq_gen(g + 1))
            if g < 4:                    # V for head groups 4..7 as filler
                parts.append(v1_gen(range(g * 4, (g + 1) * 4)))
            filler = chain(*parts)
            attention_group(g, filler)
            for _ in filler:        # drain leftovers
                pass
        s3.close()
        s23.close()

        # ---- Phase 4: proj + residual + LN2 ----
        x2T = [big.tile([P, TOWN], BF16, tag=f"Y{i}", name=f"x2T{i}")
               for i in range(FT_C)]
        h2T = [big.tile([P, TOWN], BF16, tag=f"Z{i}", name=f"h2T{i}")
               for i in range(FT_C)]
        with ExitStack() as c4:
            w4 = c4.enter_context(tc.tile_pool(name="w4", bufs=1))
            pw = _alloc(w4, KT_C, [P, C], BF16, "pw")
            for kt in range(KT_C):
                nc.gpsimd.dma_start(out=pw[kt],
                                    in_=proj_w[kt * P:(kt + 1) * P, :])
            xop = c4.enter_context(tc.tile_pool(name="xop", bufs=3))
            mm_ps4 = c4.enter_context(
                tc.tile_pool(name="mm_ps4", bufs=2, space="PSUM"))

            st4 = c4.enter_context(tc.tile_pool(name="st4", bufs=2,
                                                space="PSUM"))
            rowp4 = c4.enter_context(tc.tile_pool(name="rowp4", bufs=1))
            tpool4 = c4.enter_context(tc.tile_pool(name="tpool4", bufs=2))
            bcp4 = c4.enter_context(tc.tile_pool(name="bcp4", bufs=2))
            # nb-major: LN2 on block 0 overlaps proj of block 1
            for nb in range(NB_OWN):
                sl = slice(nb * 512, (nb + 1) * 512)
                for ft in range(FT_C):
                    pss = mm_ps4.tile([P, 512], F32, tag=f"mm{ft % 2}",
                                      name=f"prj{ft}_{nb}")
                    for kt in range(KT_C):
                        nc.tensor.matmul(
                            pss, pw[kt][:, ft * P:(ft + 1) * P],
                            attnT[kt][:, nb * 512:(nb + 1) * 512],
                            start=(kt == 0), stop=(kt == KT_C - 1))
                    t = xop.tile([P, 512], F32, tag="t4", name=f"t4{ft}_{nb}")
                    nc.vector.tensor_scalar_add(out=t, in0=pss,
                                                scalar1=projb_t[:, ft:ft + 1])
                    nc.vector.tensor_add(out=x2T[ft][:, sl], in0=t,
                                         in1=xo[ft][:, sl])
                _ln_block(nc, [x2T[kt][:, sl] for kt in range(KT_C)], h2T,
                          sl, ln2g_t, ln2b_t, eps_t, ones1, st4, rowp4,
                          tpool4, bcp4, f"l2{nb}")
        s_xt.close()

        # ---- Phase 5: MLP in two d_ff halves (SBUF-pressure): per half,
        #      h1 = relu(fc1) for 2048 dff rows, then fc2 accumulated
        #      16-deep in PSUM; halves combined in an fp32 SBUF accum ----
        NKT_H = DFF // P // 2   # 16 dff tiles per half
        with ExitStack() as c5:
            h1_pool = c5.enter_context(tc.tile_pool(name="h1_pool", bufs=1))
            oaccp = c5.enter_context(tc.tile_pool(name="oaccp", bufs=1))
            oacc = _alloc(oaccp, FT_C, [P, TOWN], F32, "oacc")
            w52 = c5.enter_context(tc.tile_pool(name="w52", bufs=1))
            w51 = c5.enter_context(tc.tile_pool(name="w51", bufs=1))
            mm_ps5 = c5.enter_context(
                tc.tile_pool(name="mm_ps5", bufs=2, space="PSUM"))
            mm_ps6 = c5.enter_context(
                tc.tile_pool(name="mm_ps6", bufs=2, space="PSUM"))
            opool = c5.enter_context(tc.tile_pool(name="opool", bufs=2))
            for dh in range(2):
                h1 = _alloc(h1_pool, NKT_H, [P, TOWN], BF16, "h1")
                w2 = [w52.tile([P, C], BF16, tag=f"w2_{i}", name=f"w2{dh}_{i}",
                               bufs=1)
                      for i in range(NKT_H)]
                for i in range(NKT_H):
                    d_ = dh * NKT_H + i
                    nc.gpsimd.dma_start(
                        out=w2[i], in_=fc2_w[d_ * P:(d_ + 1) * P, :])
                for dc in range(4):
                    w1c = [w51.tile([P, 512], BF16, tag=f"w1c{i}",
                                    name=f"w1c{dh}_{dc}_{i}", bufs=2)
                           for i in range(KT_C)]
                    for kt in range(KT_C):
                        nc.gpsimd.dma_start(
                            out=w1c[kt],
                            in_=fc1_w[kt * P:(kt + 1) * P,
                                      (dh * 4 + dc) * 512:
                                      (dh * 4 + dc + 1) * 512])
                    for m8 in range(4):
                        pss = [mm_ps5.tile([P, 512], F32, tag=f"m5{nb}",
                                           name=f"f1{dh}_{dc}_{m8}_{nb}")
                               for nb in range(NB_OWN)]
                        for kt in range(KT_C):
                            for nb in range(NB_OWN):
                                nc.tensor.matmul(
                                    pss[nb], w1c[kt][:, m8 * P:(m8 + 1) * P],
                                    h2T[kt][:, nb * 512:(nb + 1) * 512],
                                    start=(kt == 0), stop=(kt == KT_C - 1))
                        d_ = dh * NKT_H + dc * 4 + m8
                        for nb in range(NB_OWN):
                            nc.scalar.activation(
                                out=h1[dc * 4 + m8][:, nb * 512:(nb + 1) * 512],
                                in_=pss[nb], func=Relu,
                                bias=fc1b_t[:, d_:d_ + 1], scale=1.0)
                for ft in range(FT_C):
                    pss = [mm_ps6.tile([P, 512], F32, tag=f"m6{nb}",
                                       name=f"f2{dh}_{ft}_{nb}")
                           for nb in range(NB_OWN)]
                    for kt in range(NKT_H):
                        for nb in range(NB_OWN):
                            nc.tensor.matmul(
                                pss[nb], w2[kt][:, ft * P:(ft + 1) * P],
                                h1[kt][:, nb * 512:(nb + 1) * 512],
                                start=(kt == 0), stop=(kt == NKT_H - 1))
                    if dh == 0:
                        for nb in range(NB_OWN):
                            sl = slice(nb * 512, (nb + 1) * 512)
                            nc.vector.tensor_copy(out=oacc[ft][:, sl],
                                                  in_=pss[nb])
                    else:
                        o = opool.tile([P, TOWN], F32, tag="o", name=f"o{ft}")
                        for nb in range(NB_OWN):
                            sl = slice(nb * 512, (nb + 1) * 512)
                            nc.vector.tensor_add(out=o[:, sl],
                                                 in0=pss[nb],
                                                 in1=oacc[ft][:, sl])
                            nc.vector.tensor_scalar_add(
                                out=o[:, sl], in0=o[:, sl],
                                scalar1=fc2b_t[:, ft:ft + 1])
                            nc.vector.tensor_add(out=o[:, sl], in0=o[:, sl],
                                                 in1=x2T[ft][:, sl])
                        nc.sync.dma_start(out=out[ft * P:(ft + 1) * P, :],
                                          in_=o)

    nc.compile()
    return nc


_NC_CACHE = None


def _get_nc():
    global _NC_CACHE
    if _NC_CACHE is None:
        _NC_CACHE = build_nc()
    return _NC_CACHE


# permuted chunk order per core flavor j (position -> source chunk)
_PERM = {0: [1, 0, 2, 3], 1: [0, 1, 3, 2]}
_OWN_POS = (1, 3)


def _make_tri():
    # [512 keys, 512 q] lower-triangular (key visible iff k <= q), packed the
    # same way as the score tiles: row-block i holds key tiles (2i, 2i+1).
    k = np.arange(512, dtype=np.int64)[:, None]
    q = np.arange(512, dtype=np.int64)[None, :]
    m = np.where(k <= q, 0.0, NEG).astype(np.float32)   # [512k, 512q]
    return np.ascontiguousarray(m, dtype=NPBF16)


def _make_kbias(j):
    kb = np.zeros((P, 8), np.float32)
    if j == 0:
        kb[:, 0] = NEG          # qc0, key slot 0 (= chunk 1) invisible
    else:
        kb[:, 6] = NEG          # qc1, key slot 2 (= chunk 3) invisible
    return kb


def _run(inputs, trace=False):
    nc = _get_nc()
    xs = {k: np.ascontiguousarray(np.asarray(v), dtype=np.float32)
          for k, v in inputs.items()}
    x = xs["x"]
    b16 = lambda a: np.ascontiguousarray(a, dtype=NPBF16)
    attn_w16 = b16(xs["attn_w"])
    proj_w16 = b16(xs["proj_w"])
    fc1_w16 = b16(xs["fc1_w"])
    fc2_w16 = b16(xs["fc2_w"])
    tri = _make_tri()
    kbs = {j: _make_kbias(j) for j in range(2)}
    in_maps = []
    for c in range(8):
        b, j = divmod(c, 2)
        perm = _PERM[j]
        xT = x[b].T
        xT_perm = b16(np.concatenate([xT[:, p * 512:(p + 1) * 512]
                                      for p in perm], axis=1))
        in_maps.append({
            "xT_full": xT_perm,
            "tri_mask": tri,
            "kbias": kbs[j],
            "attn_w": attn_w16, "attn_b": xs["attn_b"],
            "proj_w": proj_w16, "proj_b": xs["proj_b"],
            "ln1_g": xs["ln1_g"], "ln1_b": xs["ln1_b"],
            "ln2_g": xs["ln2_g"], "ln2_b": xs["ln2_b"],
            "fc1_w": fc1_w16, "fc1_b": xs["fc1_b"],
            "fc2_w": fc2_w16, "fc2_b": xs["fc2_b"],
        })
    res = run_bass_kernel_spmd(nc, in_maps, list(range(8)), trace=trace)
    full = np.empty((B, T, C), dtype=np.float32)
    for c in range(8):
        b, j = divmod(c, 2)
        perm = _PERM[j]
        cl, ch = perm[_OWN_POS[0]], perm[_OWN_POS[1]]
        o = res.results[c]["out"]            # [C, TOWN] feature-major
        full[b, cl * 512:(cl + 1) * 512] = o[:, 0:512].T
        full[b, ch * 512:(ch + 1) * 512] = o[:, 512:1024].T
    return full, res.exec_time_ns


def kernel(**inputs):
    out, _ = _run(inputs, trace=False)
    return out


# revision 23
# speedup vs baseline: 1.0822x; 1.0096x over previous
"""Round-3 Trainium2 Bass kernel for the dense transformer block.

Key structure (vs the fp32r baseline):
- bf16 matmuls everywhere (fp32 PSUM): 1 elem/cycle moving-operand streaming
  plus separate LDWEIGHTS the PE pulls ahead of in-flight matmuls.
- Host permutes the token order per core (chunk order j=0: [1,0,2,3],
  j=1: [0,1,3,2]) so every core's OWN chunks sit at permuted positions
  {1,3}: one full-sequence LN1 feeds Q, K and V; the causal-diagonal block
  of each query chunk is at a fixed key slot (qc0 -> key tiles 4..7,
  qc1 -> 12..15), masked by one shared triangular constant. Whole-block
  visibility differences between cores are {0,-1e30} bias columns folded
  into the exp activation. No fully-general mask tensors, no per-block
  DVE mask adds outside the diagonal.
- K^T, V, x, hT all stay resident in SBUF (no DRAM spill).
- Software-pipelined emission: the attention group loop emits QK one
  iteration ahead of PV, and interleaves the NEXT group's K/Q projection
  matmuls as PE filler so the tensor engine never idles while the exp
  (ACT) of the current tile is in flight -- this both hides the
  mask->exp latency and keeps the PE HAM clock-gate at 2.4 GHz.
- MLP: all of h1 = relu(fc1) first, then fc2 accumulates the full
  4096-deep contraction in PSUM (no fp32 accumulation pass in SBUF).
- Softmax denominators come from a ones-column appended to V; their
  reciprocals run on SBUF copies (reciprocal_approx_fast is wrong on HW
  for PSUM inputs) batched per pass.
"""

from contextlib import ExitStack

import numpy as np
import ml_dtypes

import concourse.bacc as bacc
import concourse.bass as bass
import concourse.tile as tile
from concourse import mybir
from concourse.bass_utils import run_bass_kernel_spmd

F32 = mybir.dt.float32
BF16 = mybir.dt.bfloat16
NPBF16 = ml_dtypes.bfloat16
P = 128
B, T, C = 4, 2048, 1024
H, D = 16, 64
DFF = 4096
TOWN = 1024
EPS = 1e-5
SCALE = D ** -0.5
NEG = -1e30

KT_C = C // P
FT_C = C // P
TT_FULL = T // P
NGROUP = H // 2
NB_OWN = TOWN // 512

Ident = mybir.ActivationFunctionType.Identity
Sqrt = mybir.ActivationFunctionType.Sqrt
Exp = mybir.ActivationFunctionType.Exp
Relu = mybir.ActivationFunctionType.Relu

# own token slices in the permuted layout
OWN = [slice(512, 1024), slice(1536, 2048)]


def _alloc(pool, n, shape, dt, prefix, **kw):
    return [
        pool.tile(list(shape), dt, tag=f"{prefix}{i}", name=f"{prefix}{i}", **kw)
        for i in range(n)
    ]


def _ln_block(nc, xs, dst, sl, g_col, b_col, eps_t, ones1, st_ps, rowp, tpool,
              bcp, prefix):
    """One 512-token LayerNorm block, feature-major. xs: 8 [P,512] bf16 APs."""
    ssum = st_ps.tile([1, 512], F32, tag="ssum", name=f"{prefix}ss")
    ssq = st_ps.tile([1, 512], F32, tag="ssq", name=f"{prefix}sq")
    for kt in range(KT_C):
        nc.tensor.matmul(ssum, ones1, xs[kt],
                         start=(kt == 0), stop=(kt == KT_C - 1))
    for kt in range(KT_C):
        sq = tpool.tile([P, 512], BF16, tag="sqt", name=f"{prefix}sqt{kt}")
        nc.vector.tensor_mul(out=sq, in0=xs[kt], in1=xs[kt])
        nc.tensor.matmul(ssq, ones1, sq,
                         start=(kt == 0), stop=(kt == KT_C - 1))
    r0 = rowp.tile([1, 512], F32, tag="r0", name=f"{prefix}mu")       # mu
    nc.scalar.mul(r0, ssum, 1.0 / C)
    r1 = rowp.tile([1, 512], F32, tag="r1", name=f"{prefix}msq")      # msq->var->rs
    nc.scalar.mul(r1, ssq, 1.0 / C)
    r2 = rowp.tile([1, 512], F32, tag="r2", name=f"{prefix}mu2")      # mu^2->std
    nc.vector.tensor_mul(out=r2, in0=r0, in1=r0)
    nc.vector.tensor_sub(out=r1, in0=r1, in1=r2)
    nc.scalar.activation(out=r2, in_=r1, func=Sqrt,
                         bias=eps_t[0:1, 0:1], scale=1.0)
    nc.vector.reciprocal_approx_fast(out=r1, in_=r2)
    mu16 = rowp.tile([1, 512], BF16, tag="mu16", name=f"{prefix}mu16")
    nc.vector.tensor_copy(out=mu16, in_=r0)
    rs16 = rowp.tile([1, 512], BF16, tag="rs16", name=f"{prefix}rs16")
    nc.vector.tensor_copy(out=rs16, in_=r1)
    mu_b = bcp.tile([P, 512], BF16, tag="mub", name=f"{prefix}mub")
    nc.gpsimd.partition_broadcast(mu_b, mu16)
    rs_b = bcp.tile([P, 512], BF16, tag="rsb", name=f"{prefix}rsb")
    nc.gpsimd.partition_broadcast(rs_b, rs16)
    for ft in range(FT_C):
        t = tpool.tile([P, 512], BF16, tag="ap", name=f"{prefix}ap{ft}")
        nc.vector.tensor_sub(out=t, in0=xs[ft], in1=mu_b)
        nc.vector.tensor_mul(out=t, in0=t, in1=rs_b)
        nc.scalar.activation(out=dst[ft][:, sl], in_=t, func=Ident,
                             bias=b_col[:, ft:ft + 1],
                             scale=g_col[:, ft:ft + 1])


def build_nc():
    nc = bacc.Bacc()
    xT_full = nc.declare_dram_parameter("xT_full", [C, T], BF16, isOutput=False)
    tri_mask = nc.declare_dram_parameter("tri_mask", [512, 512], BF16,
                                         isOutput=False)
    kbias = nc.declare_dram_parameter("kbias", [P, 8], F32, isOutput=False)
    attn_w = nc.declare_dram_parameter("attn_w", [C, 3 * C], BF16, isOutput=False)
    attn_b = nc.declare_dram_parameter("attn_b", [3 * C], F32, isOutput=False)
    proj_w = nc.declare_dram_parameter("proj_w", [C, C], BF16, isOutput=False)
    proj_b = nc.declare_dram_parameter("proj_b", [C], F32, isOutput=False)
    ln1_g = nc.declare_dram_parameter("ln1_g", [C], F32, isOutput=False)
    ln1_b = nc.declare_dram_parameter("ln1_b", [C], F32, isOutput=False)
    ln2_g = nc.declare_dram_parameter("ln2_g", [C], F32, isOutput=False)
    ln2_b = nc.declare_dram_parameter("ln2_b", [C], F32, isOutput=False)
    fc1_w = nc.declare_dram_parameter("fc1_w", [C, DFF], BF16, isOutput=False)
    fc1_b = nc.declare_dram_parameter("fc1_b", [DFF], F32, isOutput=False)
    fc2_w = nc.declare_dram_parameter("fc2_w", [DFF, C], BF16, isOutput=False)
    fc2_b = nc.declare_dram_parameter("fc2_b", [C], F32, isOutput=False)
    out = nc.declare_dram_parameter("out", [C, TOWN], F32, isOutput=True)

    with tile.TileContext(nc) as tc, ExitStack() as top:
        const = top.enter_context(tc.tile_pool(name="const", bufs=1))
        eps_t = const.tile([P, 1], F32, name="eps_t")
        nc.vector.memset(eps_t, EPS)
        ones1 = const.tile([P, 1], BF16, name="ones1")
        nc.vector.memset(ones1, 1.0)
        ones16 = const.tile([P, H], BF16, name="ones16")
        nc.vector.memset(ones16, 1.0)
        ln1g_t = const.tile([P, FT_C], F32, name="ln1g_t")
        ln1b_t = const.tile([P, FT_C], F32, name="ln1b_t")
        ln2g_t = const.tile([P, FT_C], F32, name="ln2g_t")
        ln2b_t = const.tile([P, FT_C], F32, name="ln2b_t")
        nc.sync.dma_start(out=ln1g_t, in_=ln1_g.rearrange("(f p) -> p f", p=P))
        nc.sync.dma_start(out=ln1b_t, in_=ln1_b.rearrange("(f p) -> p f", p=P))
        nc.sync.dma_start(out=ln2g_t, in_=ln2_g.rearrange("(f p) -> p f", p=P))
        nc.sync.dma_start(out=ln2b_t, in_=ln2_b.rearrange("(f p) -> p f", p=P))
        abq_t = const.tile([P, NGROUP], F32, name="abq_t")
        abk_t = const.tile([P, NGROUP], F32, name="abk_t")
        nc.sync.dma_start(out=abq_t, in_=attn_b[0:C].rearrange("(g p) -> p g", p=P))
        nc.sync.dma_start(out=abk_t,
                          in_=attn_b[C:2 * C].rearrange("(g p) -> p g", p=P))
        projb_t = const.tile([P, FT_C], F32, name="projb_t")
        nc.sync.dma_start(out=projb_t, in_=proj_b.rearrange("(f p) -> p f", p=P))
        fc2b_t = const.tile([P, FT_C], F32, name="fc2b_t")
        nc.sync.dma_start(out=fc2b_t, in_=fc2_b.rearrange("(f p) -> p f", p=P))
        fc1b_t = const.tile([P, DFF // P], F32, name="fc1b_t")
        nc.sync.dma_start(out=fc1b_t, in_=fc1_b.rearrange("(f p) -> p f", p=P))
        kb_t = const.tile([P, 8], F32, name="kb_t")
        nc.sync.dma_start(out=kb_t, in_=kbias[:, :])
        bv_bc = const.tile([P, C], F32, name="bv_bc")
        abv = attn_b[2 * C:3 * C]
        nc.sync.dma_start(
            out=bv_bc,
            in_=bass.AP(tensor=abv.tensor, offset=abv.offset,
                        ap=[[0, P]] + list(abv.ap[-1:])))

        # HAM warm-up: a burst of dummy matmuls on a memset tile flips the
        # PE clock-gate to 8/8 while the initial DMAs are still in flight.
        with ExitStack() as cw:
            warm = cw.enter_context(tc.tile_pool(name="warm", bufs=1))
            wsrc = warm.tile([P, 512], BF16, name="wsrc")
            nc.vector.memset(wsrc, 0.0)
            wps = cw.enter_context(tc.tile_pool(name="wps", bufs=1,
                                                space="PSUM"))
            wdst = wps.tile([P, 512], F32, name="wdst")
            for i in range(24):
                nc.tensor.matmul(wdst, wsrc[:, 0:P], wsrc,
                                 start=True, stop=True)

        big = top.enter_context(tc.tile_pool(name="big", bufs=1))
        qT = [big.tile([P, TOWN], BF16, tag=f"Y{i}", name=f"qT{i}")
              for i in range(NGROUP)]

        s_xt = ExitStack()      # closed after proj residual
        xtp = s_xt.enter_context(tc.tile_pool(name="xtp", bufs=1))
        # own-token x slices (residual + LN blocks 1/3), resident to phase 4
        xo = _alloc(xtp, KT_C, [P, TOWN], BF16, "xo")

        def x_pair(kt, col0):
            """[P, 2x512] strided view of xT_full: cols col0+{0,1024}."""
            base = xT_full[kt * P:(kt + 1) * P, :]
            return bass.AP(tensor=base.tensor,
                           offset=base.offset + col0,
                           ap=[list(base.ap[0]), [1024, 2], [1, 512]])

        for kt in range(KT_C):
            nc.scalar.dma_start(out=xo[kt].rearrange("p (b x) -> p b x", b=2),
                                in_=x_pair(kt, 512))

        s23 = ExitStack()       # hT/kT/vT: closed after attention
        hfp = s23.enter_context(tc.tile_pool(name="hfp", bufs=1))
        hT = _alloc(hfp, FT_C, [P, T], BF16, "hT")
        kvp = s23.enter_context(tc.tile_pool(name="kvp", bufs=1))
        kT = _alloc(kvp, NGROUP, [P, T], BF16, "kT")
        vT = kvp.tile([P, NGROUP, TT_FULL, 130], BF16, name="vT")

        wstream = s23.enter_context(tc.tile_pool(name="wstream", bufs=1))

        def stream_w(dram_slice, tag, name, ncols, bufs=2):
            w = wstream.tile([P, ncols], BF16, tag=tag, name=f"{name}_w",
                             bufs=bufs)
            nc.gpsimd.dma_start(out=w, in_=dram_slice)
            return w

        wvp = s23.enter_context(tc.tile_pool(name="wvp", bufs=1))
        wv_all = []
        for kt in range(KT_C):
            w = wvp.tile([P, 1024], BF16, tag=f"wv{kt}",
                         name=f"wv{kt}_w", bufs=1)
            nc.sync.dma_start(
                out=w, in_=attn_w[kt * P:(kt + 1) * P, 2 * C:2 * C + 1024])
            wv_all.append([w[:, 0:512], w[:, 512:1024]])

        # ---- Phase 1+2a: LN1 (full seq) interleaved with V ----
        with ExitStack() as c1:
            st_ps = c1.enter_context(tc.tile_pool(name="st_ps", bufs=2,
                                                  space="PSUM"))
            rowp = c1.enter_context(tc.tile_pool(name="rowp", bufs=1))
            tpool = c1.enter_context(tc.tile_pool(name="tpool", bufs=2))
            lnp = c1.enter_context(tc.tile_pool(name="lnp", bufs=1))
            bcp = c1.enter_context(tc.tile_pool(name="bcp", bufs=2))
            mm_ps = c1.enter_context(
                tc.tile_pool(name="mm_ps", bufs=2, space="PSUM"))
            def emit_v(tt):
                pss = mm_ps.tile([P, 512], F32, tag="mm0",
                                 name=f"vps{tt}_0")
                for kt in range(KT_C):
                    nc.tensor.matmul(
                        pss, hT[kt][:, tt * P:(tt + 1) * P],
                        wv_all[kt][0], start=(kt == 0),
                        stop=(kt == KT_C - 1))
                nc.vector.tensor_add(
                    out=vT[:, 0:4, tt, :].rearrange(
                        "p g (h x) -> p g h x", x=65)[:, :, :, 0:64],
                    in0=pss.rearrange("p (g h d) -> p g h d", h=2, d=64),
                    in1=bv_bc[:, 0:512].rearrange(
                        "p (g h d) -> p g h d", h=2, d=64))
                nc.vector.tensor_copy(
                    out=vT[:, :, tt, 64::65].rearrange("p g h -> p g h"),
                    in_=ones16.rearrange("p (g h) -> p g h", h=2))

            # blocks 0 and 2 (non-own) preloaded as one strided DMA per kt
            x02 = _alloc(lnp, KT_C, [P, 1024], BF16, "x02")
            for kt in range(KT_C):
                nc.gpsimd.dma_start(
                    out=x02[kt].rearrange("p (b x) -> p b x", b=2),
                    in_=x_pair(kt, 0))

            def ln_inputs(nb):
                if nb == 1:
                    return [xo[kt][:, 0:512] for kt in range(KT_C)]
                if nb == 3:
                    return [xo[kt][:, 512:1024] for kt in range(KT_C)]
                half = nb // 2
                return [x02[kt][:, half * 512:(half + 1) * 512]
                        for kt in range(KT_C)]

            for nb in range(4):
                sl = slice(nb * 512, (nb + 1) * 512)
                _ln_block(nc, ln_inputs(nb), hT, sl, ln1g_t, ln1b_t, eps_t,
                          ones1, st_ps, rowp, tpool, bcp, f"lf{nb}")
                if nb >= 1:
                    for tt in range((nb - 1) * 4, nb * 4):
                        emit_v(tt)
            for tt in range(12, 16):
                emit_v(tt)

        # ---- Phase 2b/3: K/Q per group, software-pipelined into the
        #      previous group's attention ----
        attnT = [big.tile([P, TOWN], BF16, tag=f"Z{i}", name=f"attnT{i}")
                 for i in range(FT_C)]
        s3 = ExitStack()
        kq_ps = s3.enter_context(tc.tile_pool(name="kq_ps", bufs=1,
                                              space="PSUM"))
        sc_ps = s3.enter_context(tc.tile_pool(name="sc_ps", bufs=2,
                                              space="PSUM"))
        y_ps_pool = s3.enter_context(tc.tile_pool(name="y_ps", bufs=1,
                                                  space="PSUM"))
        ppool = s3.enter_context(tc.tile_pool(name="ppool", bufs=4))
        npool = s3.enter_context(tc.tile_pool(name="npool", bufs=1))
        mpool = s3.enter_context(tc.tile_pool(name="mpool", bufs=1))
        tri = _alloc(mpool, 4, [P, 512], BF16, "tri")
        for i in range(4):
            nc.gpsimd.dma_start(out=tri[i], in_=tri_mask[i * P:(i + 1) * P, :])

        def kq_gen(g):
            """Generator emitting K_g then Q_g in small steps (PE filler)."""
            wk_g = [stream_w(
                attn_w[kt * P:(kt + 1) * P, C + g * P:C + (g + 1) * P],
                f"wk{kt}", f"wk{g}_{kt}", P) for kt in range(KT_C)]
            for half in range(2):
                pss = [kq_ps.tile([P, 512], F32, tag=f"kq{nb}",
                                  name=f"kps{g}_{half}_{nb}")
                       for nb in range(2)]
                for kt in range(KT_C):
                    for nb in range(2):
                        nc.tensor.matmul(
                            pss[nb], wk_g[kt],
                            hT[kt][:, half * 1024 + nb * 512:
                                   half * 1024 + (nb + 1) * 512],
                            start=(kt == 0), stop=(kt == KT_C - 1))
                    yield
                for nb in range(2):
                    nc.vector.tensor_scalar_add(
                        out=kT[g][:, half * 1024 + nb * 512:
                                  half * 1024 + (nb + 1) * 512],
                        in0=pss[nb], scalar1=abk_t[:, g:g + 1])
                yield
            wq_g = [stream_w(
                attn_w[kt * P:(kt + 1) * P, g * P:(g + 1) * P],
                f"wq{kt}", f"wq{g}_{kt}", P) for kt in range(KT_C)]
            pss = [kq_ps.tile([P, 512], F32, tag=f"kq{nb}",
                              name=f"qps{g}_{nb}") for nb in range(2)]
            for kt in range(KT_C):
                for nb in range(NB_OWN):
                    nc.tensor.matmul(
                        pss[nb], wq_g[kt], hT[kt][:, OWN[nb]],
                        start=(kt == 0), stop=(kt == KT_C - 1))
                yield
            for nb in range(NB_OWN):
                nc.vector.tensor_scalar_add(
                    out=qT[g][:, nb * 512:(nb + 1) * 512], in0=pss[nb],
                    scalar1=abq_t[:, g:g + 1])
            yield

        def v1_gen(tts):
            """V for head groups 4..7 (nb=1), a few token tiles per call."""
            for tt in tts:
                pss = kq_ps.tile([P, 512], F32, tag="kq0", name=f"vps{tt}_1")
                for kt in range(KT_C):
                    nc.tensor.matmul(
                        pss, hT[kt][:, tt * P:(tt + 1) * P],
                        wv_all[kt][1], start=(kt == 0),
                        stop=(kt == KT_C - 1))
                    if kt % 4 == 3:
                        yield
                nc.vector.tensor_add(
                    out=vT[:, 4:8, tt, :].rearrange(
                        "p g (h x) -> p g h x", x=65)[:, :, :, 0:64],
                    in0=pss.rearrange("p (g h d) -> p g h d", h=2, d=64),
                    in1=bv_bc[:, 512:1024].rearrange(
                        "p (g h d) -> p g h d", h=2, d=64))
                yield

        def attention_group(g, filler, rate=2):
            def fill(n=1):
                for _ in range(n):
                    if next(filler, None) is None:
                        break

            for qc in range(2):          # pass A: qc0 (k2<4), pass B: qc1
                nk2 = 4 if qc == 0 else 8
                y_ps = {hh: y_ps_pool.tile([65, 512], F32, tag=f"yh{hh}",
                                           name=f"y{g}_{qc}_{hh}")
                        for hh in range(2)}
                iters = [(k2, hh) for k2 in range(nk2) for hh in range(2)]

                def emit_qk(k2, hh):
                    hsl = slice(64 * hh, 64 * (hh + 1))
                    sc = sc_ps.tile([P, 1024], F32, tag="sc",
                                    name=f"sc{g}_{qc}_{k2}_{hh}")
                    for j in range(2):
                        kt = 2 * k2 + j
                        nc.tensor.matmul(
                            sc[:, j * 512:(j + 1) * 512],
                            kT[g][hsl, kt * P:(kt + 1) * P],
                            qT[g][hsl, qc * 512:(qc + 1) * 512],
                            start=True, stop=True,
                            tile_position=(64 * hh, 0))
                    return sc

                def emit_rest(k2, hh, sc):
                    diag = (2, 3) if qc == 0 else (6, 7)
                    if k2 in diag:
                        for j in range(2):
                            nc.vector.tensor_add(
                                out=sc[:, j * 512:(j + 1) * 512],
                                in0=sc[:, j * 512:(j + 1) * 512],
                                in1=tri[2 * (k2 - diag[0]) + j])
                    pt = ppool.tile([P, 1024], BF16, tag="pt",
                                    name=f"p{g}_{qc}_{k2}_{hh}")
                    nc.scalar.activation(
                        out=pt, in_=sc, func=Exp, scale=SCALE,
                        bias=kb_t[:, qc * 4 + k2 // 2:qc * 4 + k2 // 2 + 1])
                    for j in range(2):
                        kt = 2 * k2 + j
                        nc.tensor.matmul(
                            y_ps[hh], vT[:, g, kt, 65 * hh:65 * (hh + 1)],
                            pt[:, j * 512:(j + 1) * 512],
                            start=(kt == 0), stop=(kt == 2 * nk2 - 1))

                sc_prev = emit_qk(*iters[0])
                for i, it in enumerate(iters):
                    if i + 1 < len(iters):
                        sc_next = emit_qk(*iters[i + 1])
                    fill(rate)
                    emit_rest(*it, sc_prev)
                    if i + 1 < len(iters):
                        sc_prev = sc_next

                # normalize this pass: denominators -> SBUF, recip, scale
                for hh in range(2):
                    dn = npool.tile([1, 512], F32, tag=f"dn{hh}",
                                    name=f"dn{g}_{qc}_{hh}")
                    nc.vector.tensor_copy(out=dn, in_=y_ps[hh][64:65, :])
                    rc = npool.tile([1, 512], F32, tag=f"rc{hh}",
                                    name=f"rc{g}_{qc}_{hh}")
                    nc.vector.reciprocal_approx_fast(out=rc, in_=dn)
                    r16 = npool.tile([1, 512], BF16, tag=f"r16{hh}",
                                     name=f"r16{g}_{qc}_{hh}")
                    nc.vector.tensor_copy(out=r16, in_=rc)
                    rb = npool.tile([64, 512], BF16, tag=f"rb{hh}",
                                    name=f"rb{g}_{qc}_{hh}")
                    nc.gpsimd.partition_broadcast(rb, r16)
                    nc.vector.tensor_mul(
                        out=attnT[g][64 * hh:64 * (hh + 1),
                                     qc * 512:(qc + 1) * 512],
                        in0=y_ps[hh][0:64, :], in1=rb)
                fill(1)

        from itertools import chain
        # K_0/Q_0 up front
        for _ in kq_gen(0):
            pass
        for g in range(NGROUP):
            parts = []
            if g + 1 < NGROUP:
                parts.append(kq_gen(g + 1))
            if g < 4:                    # V for head groups 4..7 as filler
                parts.append(v1_gen(range(g * 4, (g + 1) * 4)))
            filler = chain(*parts)
            attention_group(g, filler, rate=2 if g < 4 else 1)
            for _ in filler:        # drain leftovers
                pass
        s3.close()
        s23.close()

        # ---- Phase 4: proj + residual + LN2 ----
        x2T = [big.tile([P, TOWN], BF16, tag=f"Y{i}", name=f"x2T{i}")
               for i in range(FT_C)]
        h2T = [big.tile([P, TOWN], BF16, tag=f"Z{i}", name=f"h2T{i}")
               for i in range(FT_C)]
        with ExitStack() as c4:
            w4 = c4.enter_context(tc.tile_pool(name="w4", bufs=1))
            pw = _alloc(w4, KT_C, [P, C], BF16, "pw")
            for kt in range(KT_C):
                nc.gpsimd.dma_start(out=pw[kt],
                                    in_=proj_w[kt * P:(kt + 1) * P, :])
            xop = c4.enter_context(tc.tile_pool(name="xop", bufs=3))
            mm_ps4 = c4.enter_context(
                tc.tile_pool(name="mm_ps4", bufs=2, space="PSUM"))

            st4 = c4.enter_context(tc.tile_pool(name="st4", bufs=2,
                                                space="PSUM"))
            rowp4 = c4.enter_context(tc.tile_pool(name="rowp4", bufs=1))
            tpool4 = c4.enter_context(tc.tile_pool(name="tpool4", bufs=2))
            bcp4 = c4.enter_context(tc.tile_pool(name="bcp4", bufs=2))
            # nb-major: LN2 on block 0 overlaps proj of block 1
            for nb in range(NB_OWN):
                sl = slice(nb * 512, (nb + 1) * 512)
                for ft in range(FT_C):
                    pss = mm_ps4.tile([P, 512], F32, tag=f"mm{ft % 2}",
                                      name=f"prj{ft}_{nb}")
                    for kt in range(KT_C):
                        nc.tensor.matmul(
                            pss, pw[kt][:, ft * P:(ft + 1) * P],
                            attnT[kt][:, nb * 512:(nb + 1) * 512],
                            start=(kt == 0), stop=(kt == KT_C - 1))
                    t = xop.tile([P, 512], F32, tag="t4", name=f"t4{ft}_{nb}")
                    nc.vector.tensor_scalar_add(out=t, in0=pss,
                                                scalar1=projb_t[:, ft:ft + 1])
                    nc.vector.tensor_add(out=x2T[ft][:, sl], in0=t,
                                         in1=xo[ft][:, sl])
                _ln_block(nc, [x2T[kt][:, sl] for kt in range(KT_C)], h2T,
                          sl, ln2g_t, ln2b_t, eps_t, ones1, st4, rowp4,
                          tpool4, bcp4, f"l2{nb}")
        s_xt.close()

        # ---- Phase 5: MLP in two d_ff halves (SBUF-pressure): per half,
        #      h1 = relu(fc1) for 2048 dff rows, then fc2 accumulated
        #      16-deep in PSUM; halves combined in an fp32 SBUF accum ----
        NKT_H = DFF // P // 2   # 16 dff tiles per half
        with ExitStack() as c5:
            h1_pool = c5.enter_context(tc.tile_pool(name="h1_pool", bufs=1))
            oaccp = c5.enter_context(tc.tile_pool(name="oaccp", bufs=1))
            oacc = _alloc(oaccp, FT_C, [P, TOWN], F32, "oacc")
            w52 = c5.enter_context(tc.tile_pool(name="w52", bufs=1))
            w51 = c5.enter_context(tc.tile_pool(name="w51", bufs=1))
            mm_ps5 = c5.enter_context(
                tc.tile_pool(name="mm_ps5", bufs=2, space="PSUM"))
            mm_ps6 = c5.enter_context(
                tc.tile_pool(name="mm_ps6", bufs=2, space="PSUM"))
            opool = c5.enter_context(tc.tile_pool(name="opool", bufs=2))
            for dh in range(2):
                h1 = _alloc(h1_pool, NKT_H, [P, TOWN], BF16, "h1")
                w2 = [w52.tile([P, C], BF16, tag=f"w2_{i}", name=f"w2{dh}_{i}",
                               bufs=1)
                      for i in range(NKT_H)]
                for i in range(NKT_H):
                    d_ = dh * NKT_H + i
                    nc.gpsimd.dma_start(
                        out=w2[i], in_=fc2_w[d_ * P:(d_ + 1) * P, :])
                for dc in range(4):
                    w1c = [w51.tile([P, 512], BF16, tag=f"w1c{i}",
                                    name=f"w1c{dh}_{dc}_{i}", bufs=2)
                           for i in range(KT_C)]
                    for kt in range(KT_C):
                        nc.gpsimd.dma_start(
                            out=w1c[kt],
                            in_=fc1_w[kt * P:(kt + 1) * P,
                                      (dh * 4 + dc) * 512:
                                      (dh * 4 + dc + 1) * 512])
                    for m8 in range(4):
                        pss = [mm_ps5.tile([P, 512], F32, tag=f"m5{nb}",
                                           name=f"f1{dh}_{dc}_{m8}_{nb}")
                               for nb in range(NB_OWN)]
                        for kt in range(KT_C):
                            for nb in range(NB_OWN):
                                nc.tensor.matmul(
                                    pss[nb], w1c[kt][:, m8 * P:(m8 + 1) * P],
                                    h2T[kt][:, nb * 512:(nb + 1) * 512],
                                    start=(kt == 0), stop=(kt == KT_C - 1))
                        d_ = dh * NKT_H + dc * 4 + m8
                        for nb in range(NB_OWN):
                            nc.scalar.activation(
                                out=h1[dc * 4 + m8][:, nb * 512:(nb + 1) * 512],
                                in_=pss[nb], func=Relu,
                                bias=fc1b_t[:, d_:d_ + 1], scale=1.0)
                for ft in range(FT_C):
                    pss = [mm_ps6.tile([P, 512], F32, tag=f"m6{nb}",
                                       name=f"f2{dh}_{ft}_{nb}")
                           for nb in range(NB_OWN)]
                    for kt in range(NKT_H):
                        for nb in range(NB_OWN):
                            nc.tensor.matmul(
                                pss[nb], w2[kt][:, ft * P:(ft + 1) * P],
                                h1[kt][:, nb * 512:(nb + 1) * 512],
                                start=(kt == 0), stop=(kt == NKT_H - 1))
                    if dh == 0:
                        for nb in range(NB_OWN):
                            sl = slice(nb * 512, (nb + 1) * 512)
                            nc.vector.tensor_copy(out=oacc[ft][:, sl],
                                                  in_=pss[nb])
                    else:
                        o = opool.tile([P, TOWN], F32, tag="o", name=f"o{ft}")
                        for nb in range(NB_OWN):
                            sl = slice(nb * 512, (nb + 1) * 512)
                            nc.vector.tensor_add(out=o[:, sl],
                                                 in0=pss[nb],
                                                 in1=oacc[ft][:, sl])
                            nc.vector.tensor_scalar_add(
                                out=o[:, sl], in0=o[:, sl],
                                scalar1=fc2b_t[:, ft:ft + 1])
                            nc.vector.tensor_add(out=o[:, sl], in0=o[:, sl],
                                                 in1=x2T[ft][:, sl])
                        nc.sync.dma_start(out=out[ft * P:(ft + 1) * P, :],
                                          in_=o)

    nc.compile()
    return nc


_NC_CACHE = None


def _get_nc():
    global _NC_CACHE
    if _NC_CACHE is None:
        _NC_CACHE = build_nc()
    return _NC_CACHE


# permuted chunk order per core flavor j (position -> source chunk)
_PERM = {0: [1, 0, 2, 3], 1: [0, 1, 3, 2]}
_OWN_POS = (1, 3)


def _make_tri():
    # [512 keys, 512 q] lower-triangular (key visible iff k <= q), packed the
    # same way as the score tiles: row-block i holds key tiles (2i, 2i+1).
    k = np.arange(512, dtype=np.int64)[:, None]
    q = np.arange(512, dtype=np.int64)[None, :]
    m = np.where(k <= q, 0.0, NEG).astype(np.float32)   # [512k, 512q]
    return np.ascontiguousarray(m, dtype=NPBF16)


def _make_kbias(j):
    kb = np.zeros((P, 8), np.float32)
    if j == 0:
        kb[:, 0] = NEG          # qc0, key slot 0 (= chunk 1) invisible
    else:
        kb[:, 6] = NEG          # qc1, key slot 2 (= chunk 3) invisible
    return kb


def _run(inputs, trace=False):
    nc = _get_nc()
    xs = {k: np.ascontiguousarray(np.asarray(v), dtype=np.float32)
          for k, v in inputs.items()}
    x = xs["x"]
    b16 = lambda a: np.ascontiguousarray(a, dtype=NPBF16)
    attn_w16 = b16(xs["attn_w"])
    proj_w16 = b16(xs["proj_w"])
    fc1_w16 = b16(xs["fc1_w"])
    fc2_w16 = b16(xs["fc2_w"])
    tri = _make_tri()
    kbs = {j: _make_kbias(j) for j in range(2)}
    in_maps = []
    for c in range(8):
        b, j = divmod(c, 2)
        perm = _PERM[j]
        xT = x[b].T
        xT_perm = b16(np.concatenate([xT[:, p * 512:(p + 1) * 512]
                                      for p in perm], axis=1))
        in_maps.append({
            "xT_full": xT_perm,
            "tri_mask": tri,
            "kbias": kbs[j],
            "attn_w": attn_w16, "attn_b": xs["attn_b"],
            "proj_w": proj_w16, "proj_b": xs["proj_b"],
            "ln1_g": xs["ln1_g"], "ln1_b": xs["ln1_b"],
            "ln2_g": xs["ln2_g"], "ln2_b": xs["ln2_b"],
            "fc1_w": fc1_w16, "fc1_b": xs["fc1_b"],
            "fc2_w": fc2_w16, "fc2_b": xs["fc2_b"],
        })
    res = run_bass_kernel_spmd(nc, in_maps, list(range(8)), trace=trace)
    full = np.empty((B, T, C), dtype=np.float32)
    for c in range(8):
        b, j = divmod(c, 2)
        perm = _PERM[j]
        cl, ch = perm[_OWN_POS[0]], perm[_OWN_POS[1]]
        o = res.results[c]["out"]            # [C, TOWN] feature-major
        full[b, cl * 512:(cl + 1) * 512] = o[:, 0:512].T
        full[b, ch * 512:(ch + 1) * 512] = o[:, 512:1024].T
    return full, res.exec_time_ns


def kernel(**inputs):
    out, _ = _run(inputs, trace=False)
    return out
